# revision 1
# baseline (speedup 1.0000x reference)
"""Trainium2 Bass kernel for DigitConvolutionalModel.

Computes, for x [32768, 784] viewed as 28x28 images:
    feat = relu(conv3x3_valid(x))      # [B, 676]
    out  = feat @ W + b                # [B, 10]

Strategy (pure data parallel over 8 cores, 4096 rows each):
  - Host pre-transposes each core's shard to pixel-major xh [784, 4096]
    so the contraction dims sit on SBUF partitions (TensorE contracts
    partitions only).
  - Loads cast fp32 -> bf16 in the DMA datapath (SWDGE); HBM still reads
    the full fp32 input, SBUF holds bf16.
  - The 3x3 conv is a banded matmul y^T = C^T @ x^T using two constant
    blocks C1/C2 [112, 128] built on host from conv_w: input rows are
    tiled 4 image rows (112 pixels) per partition group, output rows
    4 conv rows (104 pixels, padded to 128 so weight loads qualify for
    Fast Weight Load) per PSUM tile.
  - ReLU evacuates PSUM -> SBUF bf16 (split between ScalarE and VectorE).
  - The 676->10 linear layer contracts the same pixel tiles against
    host-packed W blocks, accumulating out^T [10, chunk] fp32 in PSUM;
    bias is added during the PSUM->SBUF copy.
  - Device emits out^T [10, 4096]; host transposes back.
  - Batch is processed in column chunks: small chunks at both ends (fast
    pipeline ramp, short trailing compute), 512 in the middle (PSUM bank
    limit).

Walrus accepts only ONE semaphore wait per engine instruction, so the
kernel is arranged so every instruction needs at most one: constants are
pre-touched by tiny warm-up ops, each chunk's x-tile DMA is absorbed by
a touch matmul, redundant same-engine waits Tile emits are stripped, and
the kernel-tail drain is split into single-wait drains.
"""

import numpy as np

try:
    from concourse import bass, mybir
    from concourse.tile import TileContext
    from concourse.bass_utils import run_bass_kernel_spmd
except ImportError:  # path used when concourse is not already importable
    import sys

    sys.path.insert(0, "/opt/trn_rl_repo")
    from concourse import bass, mybir
    from concourse.tile import TileContext
    from concourse.bass_utils import run_bass_kernel_spmd

from concourse.vector_clock import ScopedClock


def _patched_drain_and_barrier(self, tick_clock, wait_clock):
    """Replacement for TileContext._drain_and_barrier: walrus rejects
    instructions carrying more than one sync wait, but the kernel-tail
    drain aggregates a wait per logical proc (~14 here). Emit a chain of
    single-wait drains on the sync queue instead."""
    nc = self.nc
    drain_inst = nc.sync.drain()
    wait_clock.add_sem_waits(
        drain_inst.ins, ScopedClock({None: tick_clock.global_clock})
    )
    si = drain_inst.ins.sync_info
    waits = list(si.on_wait or []) if si else []
    if len(waits) > 1:
        drain_inst.ins.sync_info = mybir.SyncInfo(
            on_wait=waits[:1], on_update=si.on_update
        )
        for w in waits[1:]:
            extra = nc.sync.drain()
            esi = extra.ins.sync_info
            extra.ins.sync_info = mybir.SyncInfo(
                on_wait=[w], on_update=(esi.on_update if esi else [])
            )
    nc.all_engine_barrier()
    popped = nc._tile_sem_poison_stack.pop()
    assert popped is self._sem_poison
    nc.clear_and_free_semaphores(list(self.sems.allocated().values()))
    nc.all_engine_barrier()


TileContext._drain_and_barrier = _patched_drain_and_barrier

N_CORES = 8
B = 32768
B_CORE = B // N_CORES  # 4096
# batch-chunk column windows: small chunks at the start (fast pipeline
# ramp) and at the end (short trailing compute after the last DMA lands);
# 512 (the PSUM fp32 bank limit) in the middle.
CHUNK_SIZES = [512, 512, 512, 512, 512, 512, 512, 512]
assert sum(CHUNK_SIZES) == B_CORE
OFFS = [sum(CHUNK_SIZES[:i]) for i in range(len(CHUNK_SIZES))]
NT = 7  # pixel-group tiles of 4 image rows (112 pixels); 7*4 = 28 rows

F32 = mybir.dt.float32
BF16 = mybir.dt.bfloat16
RELU = mybir.ActivationFunctionType.Relu
IDENT = mybir.ActivationFunctionType.Identity

_NC_CACHE = {}


def _build_nc():
    nc = bass.Bass(
        "TRN2", target_bir_lowering=False, debug=False, num_devices=1
    )

    # chunk-major pixel-major input: chunk n occupies rows 784n..784n+783
    # (row within chunk = pixel), cols = batch within chunk — each chunk is
    # one dense 1.6 MB block so the HBM read stream stays sequential.
    xh = nc.dram_tensor(
        "xh", [len(CHUNK_SIZES) * 784, 512], F32, kind="ExternalInput"
    )
    # packed constants: c1 | c2 | wp (columns 0:128 | 128:256 | 256:326)
    cpk_d = nc.dram_tensor("cpk", [128, 326], BF16, kind="ExternalInput")
    bias_d = nc.dram_tensor("bias_in", [10, 1], F32, kind="ExternalInput")
    out_t = nc.dram_tensor("out_t", [10, B_CORE], F32, kind="ExternalOutput")

    with TileContext(nc) as tc:
        with (
            tc.tile_pool(name="const", bufs=1) as cpool,
            tc.tile_pool(name="xc", bufs=1) as xpool,
            tc.tile_pool(name="ry_a", bufs=4) as rypool_a,
            tc.tile_pool(name="ry_v", bufs=4) as rypool_v,
            tc.tile_pool(name="outT", bufs=1) as opool,
            tc.tile_pool(name="yps_a", bufs=2, space="PSUM") as ypool_a,
            tc.tile_pool(name="yps_v", bufs=2, space="PSUM") as ypool_v,
            tc.tile_pool(name="warmp", bufs=2, space="PSUM") as warmpool,
            tc.tile_pool(name="opsum", bufs=2, space="PSUM") as opsum,
        ):
            # splits[n]: pixel-block boundaries for chunk n's DMAs. The
            # first chunks load in pieces so the conv pipeline starts as
            # soon as the first blocks land.
            splits = {0: (0, 2, 4, NT), 1: (0, 4, NT)}

            def load_block(tile, n, lo, hi):
                size = CHUNK_SIZES[n]
                blk = bass.AP(
                    xh,
                    (784 * n + 112 * lo) * size,
                    [[size, 112], [112 * size, hi - lo], [1, size]],
                )
                nc.gpsimd.dma_start(tile[:, size * lo : size * hi], blk)

            def make_tile(n):
                return xpool.tile(
                    [112, NT * CHUNK_SIZES[n]],
                    BF16,
                    tag=f"xc{n}",
                    name=f"xc{n}",
                )

            # The tiny bias DMA goes first to absorb the SWDGE pipeline's
            # cold start; then chunk 0's first sub-load, the packed
            # constants, the rest of chunk 0, and the remaining chunks.
            bias_sb = cpool.tile([10, 1], F32, tag="bias")
            nc.gpsimd.dma_start(bias_sb[:], bias_d.ap())
            xc0 = make_tile(0)
            load_block(xc0, 0, 0, 2)
            cpk_sb = cpool.tile([128, 326], BF16, tag="cpk")
            nc.gpsimd.dma_start(cpk_sb[:], cpk_d.ap())
            c1_sb = cpk_sb[0:112, 0:128]
            c2_sb = cpk_sb[0:112, 128:256]
            wp_sb = cpk_sb[:, 256:326]
            load_block(xc0, 0, 2, 4)
            load_block(xc0, 0, 4, NT)
            xc = [xc0]
            for n in range(1, len(CHUNK_SIZES)):
                tile = make_tile(n)
                for lo, hi in zip(
                    splits.get(n, (0, NT)), splits.get(n, (0, NT))[1:]
                ):
                    load_block(tile, n, lo, hi)
                xc.append(tile)

            outT_sb = opool.tile([10, B_CORE], F32, tag="outT")

            # PE HAM warm-up: the PE clock-gate only lifts to 2.4 GHz after
            # ~3.4us of sustained activity. Fill the initial DMA-wait window
            # with junk matmuls so the real matmuls run warm. The memset
            # runs on the otherwise-idle VectorE (the Pool queue is busy
            # issuing DMA triggers).
            junk = cpool.tile([112, 512], BF16, tag="junk")
            nc.vector.memset(junk[:], 0.0)
            warm = warmpool.tile([8, 512], F32, tag="warm")
            warm2 = warmpool.tile([8, 512], F32, tag="warm")
            # high_priority pins the spam at the head of the PE stream.
            with tc.high_priority():
                for i in range(6):
                    nc.tensor.matmul(
                        (warm if i % 2 == 0 else warm2)[:],
                        junk[:, 0:8],
                        junk[:],
                    )

            # Pre-touch the constants with a tiny op so real instructions'
            # dependency on their DMA is satisfied by engine program order
            # (walrus allows a single sync wait per instruction).
            nc.tensor.matmul(warm[0:4, 0:4], c1_sb[:, 0:4], c1_sb[:, 0:4])
            warm_act = cpool.tile([10, 1], F32, tag="warm_act")
            nc.scalar.activation(warm_act[:], bias_sb[:], IDENT, bias=bias_sb[:])

            for n in range(len(CHUNK_SIZES)):
                size = CHUNK_SIZES[n]
                # Touch matmuls absorb this chunk's DMA waits on PE so the
                # conv matmuls only carry their PSUM-slot wait.
                for lo in splits.get(n, (0, NT))[:-1]:
                    nc.tensor.matmul(
                        warm[0:4, 0:4],
                        xc[n][:, size * lo : size * lo + 4],
                        xc[n][:, size * lo : size * lo + 4],
                    )
                rys = []
                for t in range(NT):
                    on_act = t % 2 == 0
                    yps = (ypool_a if on_act else ypool_v).tile(
                        [128, size], F32, tag="yps"
                    )
                    nc.tensor.matmul(
                        yps[:],
                        c1_sb,
                        xc[n][:, size * t : size * (t + 1)],
                        start=True,
                        stop=(t == 6),
                    )
                    if t < 6:
                        nc.tensor.matmul(
                            yps[:],
                            c2_sb,
                            xc[n][:, size * (t + 1) : size * (t + 2)],
                            start=False,
                            stop=True,
                        )
                    ry = (rypool_a if on_act else rypool_v).tile(
                        [128, size], BF16, tag="ry"
                    )
                    if on_act:
                        nc.scalar.activation(ry[:], yps[:], RELU)
                    else:
                        nc.vector.tensor_relu(ry[:], yps[:])
                    rys.append(ry)

                ops = opsum.tile([10, size], F32, tag="ops")
                for t in range(NT):
                    nc.tensor.matmul(
                        ops[:],
                        wp_sb[:, 10 * t : 10 * (t + 1)],
                        rys[t][:],
                        start=(t == 0),
                        stop=(t == 6),
                    )
                nc.scalar.activation(
                    outT_sb[:, OFFS[n] : OFFS[n] + size],
                    ops[:],
                    IDENT,
                    bias=bias_sb[:],
                )
                # Output DMAs on the otherwise-idle SP queue: writing as
                # compute finishes hides the HBM write-receipt latency of
                # all but the last chunk. Merged to 8 DMAs total so each
                # gets a fresh DMA lane (single-wait trigger).
                nc.sync.dma_start(
                    out_t.ap()[:, OFFS[n] : OFFS[n] + size],
                    outT_sb[:, OFFS[n] : OFFS[n] + size],
                )

    _strip_self_waits(nc)
    return nc


_ENGINE_SEM_PREFIX = {
    mybir.EngineType.PE: "PE_",
    mybir.EngineType.Activation: "Activation_",
    mybir.EngineType.DVE: "DVE_",
    mybir.EngineType.Pool: "Pool_",
    mybir.EngineType.SP: "SP_",
}


def _strip_self_waits(nc):
    """Drop semaphore waits an instruction holds on its OWN engine's
    completion counter. Engines execute their queue strictly in order, so
    a wait on the own-engine sem at a value covered by program order is
    redundant — but Tile still emits it, and walrus rejects compute
    instructions carrying more than one sync wait."""
    for fn in nc.m.functions:
        for blk in fn.blocks:
            for inst in blk.instructions:
                tn = type(inst).__name__
                if tn in ("InstDrain", "InstEventSemaphore", "InstDMACopy"):
                    continue
                si = inst.sync_info
                if si is None or not si.on_wait or len(si.on_wait) < 2:
                    continue
                pref = _ENGINE_SEM_PREFIX.get(inst.engine)
                if pref is None:
                    continue
                kept = [w for w in si.on_wait if not w.ant_name.startswith(pref)]
                if len(kept) != len(si.on_wait):
                    inst.sync_info = mybir.SyncInfo(
                        on_wait=kept, on_update=si.on_update
                    )


def _build_consts(conv_w, W, b):
    conv_w = np.asarray(conv_w, np.float32)
    W = np.asarray(W, np.float32)
    b = np.asarray(b, np.float32)

    # C1: input rows 4t+rl (rl 0..3) -> output conv rows 4t+il (il 0..3)
    # C2: input rows 4(t+1)+rl      -> output conv rows 4t+il
    c1 = np.zeros((112, 128), np.float32)
    c2 = np.zeros((112, 128), np.float32)
    for rl in range(4):
        for c in range(28):
            for il in range(4):
                for j in range(26):
                    dj = c - j
                    if not (0 <= dj <= 2):
                        continue
                    di = rl - il
                    if 0 <= di <= 2:
                        c1[rl * 28 + c, il * 26 + j] = conv_w[di, dj]
                    di2 = 4 + rl - il
                    if 0 <= di2 <= 2:
                        c2[rl * 28 + c, il * 26 + j] = conv_w[di2, dj]

    # W packed: block t holds rows for conv-output rows 4t..4t+3
    wp = np.zeros((128, 70), np.float32)
    for t in range(6):
        wp[0:104, 10 * t : 10 * (t + 1)] = W[104 * t : 104 * (t + 1)]
    wp[0:52, 60:70] = W[624:676]

    import ml_dtypes

    cpk = np.zeros((128, 326), np.float32)
    cpk[0:112, 0:128] = c1
    cpk[0:112, 128:256] = c2
    cpk[:, 256:326] = wp
    return cpk.astype(ml_dtypes.bfloat16), b.reshape(10, 1).copy()


def _run(inputs, trace=False):
    x = np.asarray(inputs["x"], np.float32)
    conv_w = inputs["conv_w"]
    W = inputs["W"]
    b = inputs["b"]

    if "nc" not in _NC_CACHE:
        _NC_CACHE["nc"] = _build_nc()
    nc = _NC_CACHE["nc"]

    cpk, bias = _build_consts(conv_w, W, b)

    in_maps = []
    for c in range(N_CORES):
        shard = x[c * B_CORE : (c + 1) * B_CORE]  # [4096, 784]
        # [8, 512, 784] -> [8, 784, 512]: chunk-major, pixel rows
        xh = np.ascontiguousarray(
            shard.reshape(len(CHUNK_SIZES), 512, 784).transpose(0, 2, 1)
        ).reshape(len(CHUNK_SIZES) * 784, 512)
        in_maps.append({"xh": xh, "cpk": cpk, "bias_in": bias})

    res = run_bass_kernel_spmd(
        nc, in_maps, core_ids=list(range(N_CORES)), trace=trace
    )
    out = np.concatenate(
        [np.asarray(res.results[c]["out_t"]).T for c in range(N_CORES)], axis=0
    )
    return out, res


def kernel(**inputs) -> np.ndarray:
    return _run(inputs, trace=False)[0]



# revision 4
# speedup vs baseline: 1.0837x; 1.0837x over previous
"""Trainium2 Bass kernel for DigitConvolutionalModel.

Computes, for x [32768, 784] viewed as 28x28 images:
    feat = relu(conv3x3_valid(x))      # [B, 676]
    out  = feat @ W + b                # [B, 10]

Strategy (pure data parallel over 8 cores, 4096 rows each):
  - Host pre-casts x to bf16 and relays each core's shard into a
    chunk-major, partition-contiguous layout: chunk n (512 samples) is
    [128 partitions, 7 pixel-tiles x 512 samples], pixel row 128t+p on
    partition p at columns 512t.. (rows 784..895 zero-padded). Each
    chunk loads with ONE clean HWDGE DMA (7KB contiguous per partition);
    chunk 0 is split into 7 per-tile sub-DMAs so compute starts early.
  - Conv as banded matmul over 128-pixel tiles: output tile u (128
    packed valid outputs) = C1_u^T @ xtile_u + C2_u^T @ xtile_{u+1},
    with C1/C2 [128,128] constants built on host from conv_w.
    12 conv matmuls per chunk; relu evacuates PSUM -> SBUF bf16
    alternating ScalarE/VectorE.
  - Linear: out^T [10,512] = sum_u wp_u^T @ ry_u (6 matmuls), bias
    added during the PSUM->SBUF copy; out^T DMAs out on the scalar
    HWDGE ring (input loads use the sync ring, so they never queue
    behind each other).
  - PE warm-up: junk matmuls spam the PE from ~0.4us so the HAM power
    grant (which needs sustained activity) arrives as early as
    possible; they also soak the initial DMA wait.

Walrus accepts only ONE semaphore wait per engine instruction, so the
kernel is arranged so every instruction needs at most one: constants
and each x DMA are pre-touched by tiny matmuls on PE, bias by a tiny
activation on ScalarE, and redundant same-engine waits are stripped.
"""

import numpy as np

try:
    from concourse import bass, mybir
    from concourse.tile import TileContext
    from concourse.bass_utils import run_bass_kernel_spmd
except ImportError:  # path used when concourse is not already importable
    import sys

    sys.path.insert(0, "/opt/trn_rl_repo")
    from concourse import bass, mybir
    from concourse.tile import TileContext
    from concourse.bass_utils import run_bass_kernel_spmd

from concourse.vector_clock import ScopedClock


def _patched_drain_and_barrier(self, tick_clock, wait_clock):
    """Replacement for TileContext._drain_and_barrier: walrus rejects
    instructions carrying more than one sync wait, but the kernel-tail
    drain aggregates a wait per logical proc. Emit a chain of
    single-wait drains on the sync queue instead."""
    nc = self.nc
    drain_inst = nc.sync.drain()
    wait_clock.add_sem_waits(
        drain_inst.ins, ScopedClock({None: tick_clock.global_clock})
    )
    si = drain_inst.ins.sync_info
    waits = list(si.on_wait or []) if si else []
    if len(waits) > 1:
        drain_inst.ins.sync_info = mybir.SyncInfo(
            on_wait=waits[:1], on_update=si.on_update
        )
        for w in waits[1:]:
            extra = nc.sync.drain()
            esi = extra.ins.sync_info
            extra.ins.sync_info = mybir.SyncInfo(
                on_wait=[w], on_update=(esi.on_update if esi else [])
            )
    nc.all_engine_barrier()
    popped = nc._tile_sem_poison_stack.pop()
    assert popped is self._sem_poison
    nc.clear_and_free_semaphores(list(self.sems.allocated().values()))
    nc.all_engine_barrier()


TileContext._drain_and_barrier = _patched_drain_and_barrier

N_CORES = 8
B = 32768
B_CORE = B // N_CORES  # 4096
N_CHUNKS = 8
CHUNK = 512  # PSUM fp32 bank limit
NT = 7  # 128-pixel input tiles (784 -> 6 full + 16-row stub, padded)
NU = 6  # output tiles of 128 packed valid conv outputs (676 total)
N_JUNK = 12  # PE warm-up matmuls before real work

F32 = mybir.dt.float32
BF16 = mybir.dt.bfloat16
RELU = mybir.ActivationFunctionType.Relu
IDENT = mybir.ActivationFunctionType.Identity

_NC_CACHE = {}


def _build_nc():
    nc = bass.Bass(
        "TRN2", target_bir_lowering=False, debug=False, num_devices=1
    )

    # chunk-major partition-contiguous input: row 128n+p holds, for
    # chunk n and partition p, the 7 pixel-tiles' samples back to back
    # (3584 bf16 = 7KB contiguous per partition line).
    xh = nc.dram_tensor(
        "xh", [N_CHUNKS * 128, NT * CHUNK], BF16, kind="ExternalInput"
    )
    # packed constants: C1_u | C2_u | wp  (cols 0:768 | 768:1536 | 1536:1596)
    cpk_d = nc.dram_tensor("cpk", [128, 1596], BF16, kind="ExternalInput")
    bias_d = nc.dram_tensor("bias_in", [10, 1], F32, kind="ExternalInput")
    out_t = nc.dram_tensor("out_t", [10, B_CORE], F32, kind="ExternalOutput")

    with TileContext(nc) as tc:
        with (
            tc.tile_pool(name="const", bufs=1) as cpool,
            tc.tile_pool(name="xc", bufs=1) as xpool,
            tc.tile_pool(name="ry_a", bufs=4) as rypool_a,
            tc.tile_pool(name="ry_v", bufs=4) as rypool_v,
            tc.tile_pool(name="outT", bufs=1) as opool,
            tc.tile_pool(name="yps_a", bufs=2, space="PSUM") as ypool_a,
            tc.tile_pool(name="yps_v", bufs=2, space="PSUM") as ypool_v,
            tc.tile_pool(name="warmp", bufs=2, space="PSUM") as warmpool,
            tc.tile_pool(name="opsum", bufs=2, space="PSUM") as opsum,
        ):
            # All loads on the SWDGE (gpsimd) queue so the 8 output
            # stores get the 8 HWDGE completion-sem lanes to themselves
            # (walrus allows only one sync wait per instruction). The
            # tiny bias DMA goes first to absorb the SWDGE cold start.
            bias_sb = cpool.tile([10, 1], F32, tag="bias")
            nc.gpsimd.dma_start(bias_sb[:], bias_d.ap())
            xc = []
            for n in range(N_CHUNKS):
                xc.append(
                    xpool.tile(
                        [128, NT * CHUNK], BF16, tag=f"xc{n}", name=f"xc{n}"
                    )
                )
            # chunk 0 split per tile so the first conv matmuls start early;
            # constants interleave after the first two sub-loads.
            for t in range(2):
                nc.gpsimd.dma_start(
                    xc[0][:, CHUNK * t : CHUNK * (t + 1)],
                    xh.ap()[0:128, CHUNK * t : CHUNK * (t + 1)],
                )
            cpk_sb = cpool.tile([128, 1596], BF16, tag="cpk")
            nc.gpsimd.dma_start(cpk_sb[:], cpk_d.ap())
            for t in range(2, NT):
                nc.gpsimd.dma_start(
                    xc[0][:, CHUNK * t : CHUNK * (t + 1)],
                    xh.ap()[0:128, CHUNK * t : CHUNK * (t + 1)],
                )
            for n in range(1, N_CHUNKS):
                nc.gpsimd.dma_start(
                    xc[n][:], xh.ap()[128 * n : 128 * (n + 1), :]
                )

            outT_sb = opool.tile([10, B_CORE], F32, tag="outT")

            # PE HAM warm-up: the PE power grant needs sustained activity.
            # Spam full-width junk matmuls from the earliest moment (only
            # gated on the VectorE memset) so the grant arrives early and
            # the initial DMA wait is soaked.
            junk = cpool.tile([128, 256], BF16, tag="junk")
            nc.vector.memset(junk[:], 0.0)
            warm = warmpool.tile([128, 128], F32, tag="warm")
            warm2 = warmpool.tile([128, 128], F32, tag="warm")
            with tc.high_priority():
                for i in range(N_JUNK):
                    nc.tensor.matmul(
                        (warm if i % 2 == 0 else warm2)[:],
                        junk[:, 0:128],
                        junk[:, 128:256],
                    )

            # Pre-touch constants on their consumer engines so real
            # instructions rely on engine program order for that DMA.
            nc.tensor.matmul(warm[0:4, 0:4], cpk_sb[:, 0:4], cpk_sb[:, 0:4])
            warm_act = cpool.tile([10, 1], F32, tag="warm_act")
            nc.scalar.activation(warm_act[:], bias_sb[:], IDENT, bias=bias_sb[:])

            c1 = lambda u: cpk_sb[:, 128 * u : 128 * (u + 1)]
            c2 = lambda u: cpk_sb[:, 768 + 128 * u : 768 + 128 * (u + 1)]
            wp = lambda u: cpk_sb[:, 1536 + 10 * u : 1536 + 10 * (u + 1)]

            def touch(n, t):
                nc.tensor.matmul(
                    warm2[0:4, 0:4],
                    xc[n][:, CHUNK * t : CHUNK * t + 4],
                    xc[n][:, CHUNK * t : CHUNK * t + 4],
                )

            for n in range(N_CHUNKS):
                off = CHUNK * n
                if n == 0:
                    touch(0, 0)
                else:
                    touch(n, 0)
                rys = []
                for u in range(NU):
                    if n == 0:
                        touch(0, u + 1)
                    on_act = u % 2 == 0
                    yps = (ypool_a if on_act else ypool_v).tile(
                        [128, CHUNK], F32, tag="yps"
                    )
                    nc.tensor.matmul(
                        yps[:],
                        c1(u),
                        xc[n][:, CHUNK * u : CHUNK * (u + 1)],
                        start=True,
                        stop=False,
                    )
                    nc.tensor.matmul(
                        yps[:],
                        c2(u),
                        xc[n][:, CHUNK * (u + 1) : CHUNK * (u + 2)],
                        start=False,
                        stop=True,
                    )
                    ry = (rypool_a if on_act else rypool_v).tile(
                        [128, CHUNK], BF16, tag="ry"
                    )
                    if on_act:
                        nc.scalar.activation(ry[:], yps[:], RELU)
                    else:
                        nc.vector.tensor_relu(ry[:], yps[:])
                    rys.append(ry)

                ops = opsum.tile([10, CHUNK], F32, tag="ops")
                for u in range(NU):
                    nc.tensor.matmul(
                        ops[:],
                        wp(u),
                        rys[u][:],
                        start=(u == 0),
                        stop=(u == NU - 1),
                    )
                nc.scalar.activation(
                    outT_sb[:, off : off + CHUNK],
                    ops[:],
                    IDENT,
                    bias=bias_sb[:],
                )
                # Output DMAs on the otherwise-idle sync HWDGE ring:
                # 8 stores total so each gets a fresh DMA lane
                # (single-wait trigger).
                nc.sync.dma_start(
                    out_t.ap()[:, off : off + CHUNK],
                    outT_sb[:, off : off + CHUNK],
                )

    _strip_self_waits(nc)
    return nc


_ENGINE_SEM_PREFIX = {
    mybir.EngineType.PE: "PE_",
    mybir.EngineType.Activation: "Activation_",
    mybir.EngineType.DVE: "DVE_",
    mybir.EngineType.Pool: "Pool_",
    mybir.EngineType.SP: "SP_",
}


def _strip_self_waits(nc):
    """Drop semaphore waits an instruction holds on its OWN engine's
    completion counter. Engines execute their queue strictly in order, so
    a wait on the own-engine sem at a value covered by program order is
    redundant — but Tile still emits it, and walrus rejects compute
    instructions carrying more than one sync wait."""
    for fn in nc.m.functions:
        for blk in fn.blocks:
            for inst in blk.instructions:
                tn = type(inst).__name__
                if tn in ("InstDrain", "InstEventSemaphore", "InstDMACopy"):
                    continue
                si = inst.sync_info
                if si is None or not si.on_wait or len(si.on_wait) < 2:
                    continue
                pref = _ENGINE_SEM_PREFIX.get(inst.engine)
                if pref is None:
                    continue
                kept = [w for w in si.on_wait if not w.ant_name.startswith(pref)]
                if len(kept) != len(si.on_wait):
                    inst.sync_info = mybir.SyncInfo(
                        on_wait=kept, on_update=si.on_update
                    )


def _build_consts(conv_w, W, b):
    conv_w = np.asarray(conv_w, np.float32)
    W = np.asarray(W, np.float32)
    b = np.asarray(b, np.float32)

    # C1_u: input tile u (pixels 128u..128u+127) -> output tile u
    # C2_u: input tile u+1 -> output tile u. Valid conv outputs are
    # packed densely: k = 26*i + j  <->  pixel offset o = 28*i + j.
    c1 = np.zeros((NU, 128, 128), np.float32)
    c2 = np.zeros((NU, 128, 128), np.float32)
    for u in range(NU):
        for m in range(128):
            k = 128 * u + m
            if k >= 676:
                break
            o = 28 * (k // 26) + (k % 26)
            for di in range(3):
                for dj in range(3):
                    p = o + 28 * di + dj
                    if 128 * u <= p < 128 * (u + 1):
                        c1[u, p - 128 * u, m] = conv_w[di, dj]
                    else:
                        c2[u, p - 128 * (u + 1), m] = conv_w[di, dj]

    wp = np.zeros((128, 60), np.float32)
    for u in range(NU):
        hi = min(128 * (u + 1), 676)
        wp[0 : hi - 128 * u, 10 * u : 10 * (u + 1)] = W[128 * u : hi]

    import ml_dtypes

    cpk = np.zeros((128, 1596), np.float32)
    cpk[:, 0:768] = c1.transpose(1, 0, 2).reshape(128, 768)
    cpk[:, 768:1536] = c2.transpose(1, 0, 2).reshape(128, 768)
    cpk[:, 1536:1596] = wp
    return cpk.astype(ml_dtypes.bfloat16), b.reshape(10, 1).copy()


def _pack_x(x):
    """[32768, 784] fp32 -> per-core [1024, 3584] bf16, chunk-major,
    partition-contiguous (row 128n+p, col 512t+c <- chunk n, pixel
    128t+p, sample c; pixels 784..895 zero)."""
    import ml_dtypes

    xb = np.ascontiguousarray(x.astype(ml_dtypes.bfloat16))
    shards = []
    for c in range(N_CORES):
        shard = xb[c * B_CORE : (c + 1) * B_CORE]  # [4096, 784]
        xp = shard.reshape(N_CHUNKS, CHUNK, 784).transpose(0, 2, 1)  # [8,784,512]
        pad = np.zeros((N_CHUNKS, 896, CHUNK), ml_dtypes.bfloat16)
        pad[:, :784] = xp
        # [8, 7, 128, 512] -> [8, 128, 7, 512] -> [1024, 3584]
        xh = np.ascontiguousarray(
            pad.reshape(N_CHUNKS, NT, 128, CHUNK).transpose(0, 2, 1, 3)
        ).reshape(N_CHUNKS * 128, NT * CHUNK)
        shards.append(xh)
    return shards


def _run(inputs, trace=False):
    x = np.asarray(inputs["x"], np.float32)
    conv_w = inputs["conv_w"]
    W = inputs["W"]
    b = inputs["b"]

    if "nc" not in _NC_CACHE:
        _NC_CACHE["nc"] = _build_nc()
    nc = _NC_CACHE["nc"]

    cpk, bias = _build_consts(conv_w, W, b)
    shards = _pack_x(x)

    in_maps = [
        {"xh": shards[c], "cpk": cpk, "bias_in": bias} for c in range(N_CORES)
    ]

    res = run_bass_kernel_spmd(
        nc, in_maps, core_ids=list(range(N_CORES)), trace=trace
    )
    out = np.concatenate(
        [np.asarray(res.results[c]["out_t"]).T for c in range(N_CORES)], axis=0
    )
    return out, res


def kernel(**inputs) -> np.ndarray:
    return _run(inputs, trace=False)[0]


# revision 5
# speedup vs baseline: 1.1169x; 1.0307x over previous
"""Trainium2 Bass kernel for DigitConvolutionalModel.

Computes, for x [32768, 784] viewed as 28x28 images:
    feat = relu(conv3x3_valid(x))      # [B, 676]
    out  = feat @ W + b                # [B, 10]

Strategy (pure data parallel over 8 cores, 4096 rows each):
  - Host pre-casts x to bf16 and relays each core's shard into a
    chunk-major, partition-contiguous layout: chunk n (512 samples) is
    [128 partitions, 7 pixel-tiles x 512 samples], pixel row 128t+p on
    partition p at columns 512t.. (rows 784..895 zero-padded), so every
    chunk loads as 128 clean 7KB-contiguous runs.
  - Chunk 0 and the packed constants load via HWDGE (sync / scalar
    rings — RTL descriptor generation starts right at preamble end);
    chunks 1-7 stream via SWDGE (gpsimd) concurrently. The 8 HWDGE
    completion-sem lanes are budgeted exactly (2 chunk-0 sub-loads +
    cpk + 4 single stores + 1 merged store for chunks 4-7) so no HWDGE
    DMA ever carries a lane-reuse wait on top of its producer wait
    (walrus allows one sync wait per instruction).
  - Conv as banded matmul over 128-pixel tiles: output tile u (128
    packed valid outputs) = C1_u^T @ xtile_u + C2_u^T @ xtile_{u+1},
    with C1/C2 [128,128] constants built on host from conv_w.
    12 conv matmuls per chunk; relu evacuates PSUM -> SBUF bf16
    alternating ScalarE/VectorE.
  - Linear: out^T [10,512] = sum_u wp_u^T @ ry_u (6 matmuls), bias
    added during the PSUM->SBUF copy.
  - PE HAM warm-up: junk matmuls spam the PE gap-free from preamble end
    until chunk 0 lands, so the HAM power grant (which needs sustained
    activity) arrives early; more junk after the last real matmul holds
    the clock up through the fixed ~50-per-engine semaphore-file reset
    that walrus appends (it runs at half clock otherwise).
"""

import numpy as np

try:
    from concourse import bass, mybir
    from concourse.tile import TileContext
    from concourse.bass_utils import run_bass_kernel_spmd
except ImportError:  # path used when concourse is not already importable
    import sys

    sys.path.insert(0, "/opt/trn_rl_repo")
    from concourse import bass, mybir
    from concourse.tile import TileContext
    from concourse.bass_utils import run_bass_kernel_spmd

from concourse.vector_clock import ScopedClock


def _patched_drain_and_barrier(self, tick_clock, wait_clock):
    """Replacement for TileContext._drain_and_barrier: walrus rejects
    instructions carrying more than one sync wait, but the kernel-tail
    drain aggregates a wait per logical proc (~25 here). Emit
    single-wait drains spread across all engine queues (they run
    concurrently) instead of one serial chain."""
    nc = self.nc
    drain_inst = nc.sync.drain()
    wait_clock.add_sem_waits(
        drain_inst.ins, ScopedClock({None: tick_clock.global_clock})
    )
    si = drain_inst.ins.sync_info
    waits = list(si.on_wait or []) if si else []
    if len(waits) > 1:
        drain_inst.ins.sync_info = mybir.SyncInfo(
            on_wait=waits[:1], on_update=si.on_update
        )
        queues = [nc.sync, nc.tensor, nc.scalar, nc.vector, nc.gpsimd]
        for i, w in enumerate(waits[1:]):
            eng = queues[i % len(queues)]
            extra = eng.drain()
            esi = extra.ins.sync_info
            extra.ins.sync_info = mybir.SyncInfo(
                on_wait=[w], on_update=(esi.on_update if esi else [])
            )
    nc.all_engine_barrier()
    popped = nc._tile_sem_poison_stack.pop()
    assert popped is self._sem_poison
    nc.clear_and_free_semaphores(list(self.sems.allocated().values()))
    nc.all_engine_barrier()


TileContext._drain_and_barrier = _patched_drain_and_barrier

N_CORES = 8
B = 32768
B_CORE = B // N_CORES  # 4096
N_CHUNKS = 8
CHUNK = 512  # PSUM fp32 bank limit
NT = 7  # 128-pixel input tiles (784 -> 6 full + 16-row stub, padded)
NU = 6  # output tiles of 128 packed valid conv outputs (676 total)
N_JUNK_HEAD = 26  # PE warm-up matmuls before real work
N_JUNK_TAIL = 30  # PE matmuls after real work to hold the HAM clock up
SPLIT0 = 3  # chunk-0 sub-load boundary (tiles [0,3) then [3,7))

F32 = mybir.dt.float32
BF16 = mybir.dt.bfloat16
RELU = mybir.ActivationFunctionType.Relu
IDENT = mybir.ActivationFunctionType.Identity

_NC_CACHE = {}


def _build_nc():
    nc = bass.Bass(
        "TRN2", target_bir_lowering=False, debug=False, num_devices=1
    )

    # chunk-major partition-contiguous input: row 128n+p holds, for
    # chunk n and partition p, the 7 pixel-tiles' samples back to back
    # (3584 bf16 = 7KB contiguous per partition line).
    xh = nc.dram_tensor(
        "xh", [N_CHUNKS * 128, NT * CHUNK], BF16, kind="ExternalInput"
    )
    # packed constants: C1_u | C2_u | wp  (cols 0:768 | 768:1536 | 1536:1596)
    cpk_d = nc.dram_tensor("cpk", [128, 1596], BF16, kind="ExternalInput")
    bias_d = nc.dram_tensor("bias_in", [10, 1], F32, kind="ExternalInput")
    out_t = nc.dram_tensor("out_t", [10, B_CORE], F32, kind="ExternalOutput")

    with TileContext(nc) as tc:
        with (
            tc.tile_pool(name="const", bufs=1) as cpool,
            tc.tile_pool(name="xc", bufs=1) as xpool,
            tc.tile_pool(name="ry_a", bufs=4) as rypool_a,
            tc.tile_pool(name="ry_v", bufs=4) as rypool_v,
            tc.tile_pool(name="outT", bufs=1) as opool,
            tc.tile_pool(name="yps_a", bufs=2, space="PSUM") as ypool_a,
            tc.tile_pool(name="yps_v", bufs=2, space="PSUM") as ypool_v,
            tc.tile_pool(name="warmp", bufs=2, space="PSUM") as warmpool,
            tc.tile_pool(name="opsum", bufs=2, space="PSUM") as opsum,
        ):
            xc = []
            for n in range(N_CHUNKS):
                xc.append(
                    xpool.tile(
                        [128, NT * CHUNK], BF16, tag=f"xc{n}", name=f"xc{n}"
                    )
                )

            # chunk 0 via HWDGE on the sync ring (2 sub-loads so conv can
            # start after the first), constants via HWDGE on the scalar
            # ring — all three start right at preamble end.
            nc.sync.dma_start(
                xc[0][:, 0 : CHUNK * SPLIT0], xh.ap()[0:128, 0 : CHUNK * SPLIT0]
            )
            nc.sync.dma_start(
                xc[0][:, CHUNK * SPLIT0 :], xh.ap()[0:128, CHUNK * SPLIT0 :]
            )
            cpk_sb = cpool.tile([128, 1596], BF16, tag="cpk")
            nc.scalar.dma_start(cpk_sb[:], cpk_d.ap())

            # bias + chunks 1-7 via SWDGE (gpsimd); the tiny bias DMA
            # absorbs the SWDGE pipeline cold start.
            bias_sb = cpool.tile([10, 1], F32, tag="bias")
            nc.gpsimd.dma_start(bias_sb[:], bias_d.ap())
            for n in range(1, N_CHUNKS):
                nc.gpsimd.dma_start(
                    xc[n][:], xh.ap()[128 * n : 128 * (n + 1), :]
                )

            outT_sb = opool.tile([10, B_CORE], F32, tag="outT")

            # PE HAM warm-up: spam junk matmuls from the earliest moment
            # (only gated on the VectorE memset) so the power grant
            # arrives early and the initial DMA wait is soaked.
            junk = cpool.tile([128, 256], BF16, tag="junk")
            nc.vector.memset(junk[:], 0.0)
            warm = warmpool.tile([128, 128], F32, tag="warm")
            warm2 = warmpool.tile([128, 128], F32, tag="warm")

            def junk_mm(i):
                nc.tensor.matmul(
                    (warm if i % 2 == 0 else warm2)[:],
                    junk[:, 0:128],
                    junk[:, 128:256],
                )

            with tc.high_priority():
                for i in range(N_JUNK_HEAD):
                    junk_mm(i)

            # Pre-touch constants on their consumer engines so real
            # instructions rely on engine program order for that DMA.
            nc.tensor.matmul(warm[0:4, 0:4], cpk_sb[:, 0:4], cpk_sb[:, 0:4])
            warm_act = cpool.tile([10, 1], F32, tag="warm_act")
            nc.scalar.activation(warm_act[:], bias_sb[:], IDENT, bias=bias_sb[:])

            c1 = lambda u: cpk_sb[:, 128 * u : 128 * (u + 1)]
            c2 = lambda u: cpk_sb[:, 768 + 128 * u : 768 + 128 * (u + 1)]
            wp = lambda u: cpk_sb[:, 1536 + 10 * u : 1536 + 10 * (u + 1)]

            def touch(n, col):
                nc.tensor.matmul(
                    warm2[0:4, 0:4],
                    xc[n][:, col : col + 4],
                    xc[n][:, col : col + 4],
                )

            for n in range(N_CHUNKS):
                off = CHUNK * n
                touch(n, 0)
                rys = []
                for u in range(NU):
                    if n == 0 and u == SPLIT0 - 1:
                        # conv u needs tile u+1: second chunk-0 sub-load
                        touch(0, CHUNK * SPLIT0)
                    on_act = u % 2 == 0
                    yps = (ypool_a if on_act else ypool_v).tile(
                        [128, CHUNK], F32, tag="yps"
                    )
                    nc.tensor.matmul(
                        yps[:],
                        c1(u),
                        xc[n][:, CHUNK * u : CHUNK * (u + 1)],
                        start=True,
                        stop=False,
                    )
                    nc.tensor.matmul(
                        yps[:],
                        c2(u),
                        xc[n][:, CHUNK * (u + 1) : CHUNK * (u + 2)],
                        start=False,
                        stop=True,
                    )
                    ry = (rypool_a if on_act else rypool_v).tile(
                        [128, CHUNK], BF16, tag="ry"
                    )
                    if on_act:
                        nc.scalar.activation(ry[:], yps[:], RELU)
                    else:
                        nc.vector.tensor_relu(ry[:], yps[:])
                    rys.append(ry)

                ops = opsum.tile([10, CHUNK], F32, tag="ops")
                for u in range(NU):
                    nc.tensor.matmul(
                        ops[:],
                        wp(u),
                        rys[u][:],
                        start=(u == 0),
                        stop=(u == NU - 1),
                    )
                nc.scalar.activation(
                    outT_sb[:, off : off + CHUNK],
                    ops[:],
                    IDENT,
                    bias=bias_sb[:],
                )
                # Stores on the sync HWDGE ring: chunks 0-3 singly, 4-7
                # as one merged DMA after chunk 7 — exactly 8 HWDGE DMAs
                # total (incl. loads), so every store has a fresh lane.
                if n < 4:
                    nc.sync.dma_start(
                        out_t.ap()[:, off : off + CHUNK],
                        outT_sb[:, off : off + CHUNK],
                    )
                elif n == N_CHUNKS - 1:
                    nc.sync.dma_start(
                        out_t.ap()[:, 4 * CHUNK :],
                        outT_sb[:, 4 * CHUNK :],
                    )

            # Hold the HAM clock up through the walrus semaphore-file
            # reset that follows (it otherwise drops to half clock ~3us
            # after the PE goes idle, doubling the fixed teardown cost).
            for i in range(N_JUNK_TAIL):
                junk_mm(i)

    _strip_self_waits(nc)
    return nc


_ENGINE_SEM_PREFIX = {
    mybir.EngineType.PE: "PE_",
    mybir.EngineType.Activation: "Activation_",
    mybir.EngineType.DVE: "DVE_",
    mybir.EngineType.Pool: "Pool_",
    mybir.EngineType.SP: "SP_",
}


def _strip_self_waits(nc):
    """Drop semaphore waits an instruction holds on its OWN engine's
    completion counter. Engines execute their queue strictly in order, so
    a wait on the own-engine sem at a value covered by program order is
    redundant — but Tile still emits it, and walrus rejects compute
    instructions carrying more than one sync wait."""
    for fn in nc.m.functions:
        for blk in fn.blocks:
            for inst in blk.instructions:
                tn = type(inst).__name__
                if tn in ("InstDrain", "InstEventSemaphore", "InstDMACopy"):
                    continue
                si = inst.sync_info
                if si is None or not si.on_wait or len(si.on_wait) < 2:
                    continue
                pref = _ENGINE_SEM_PREFIX.get(inst.engine)
                if pref is None:
                    continue
                kept = [w for w in si.on_wait if not w.ant_name.startswith(pref)]
                if len(kept) != len(si.on_wait):
                    inst.sync_info = mybir.SyncInfo(
                        on_wait=kept, on_update=si.on_update
                    )


def _build_consts(conv_w, W, b):
    conv_w = np.asarray(conv_w, np.float32)
    W = np.asarray(W, np.float32)
    b = np.asarray(b, np.float32)

    # C1_u: input tile u (pixels 128u..128u+127) -> output tile u
    # C2_u: input tile u+1 -> output tile u. Valid conv outputs are
    # packed densely: k = 26*i + j  <->  pixel offset o = 28*i + j.
    c1 = np.zeros((NU, 128, 128), np.float32)
    c2 = np.zeros((NU, 128, 128), np.float32)
    for u in range(NU):
        for m in range(128):
            k = 128 * u + m
            if k >= 676:
                break
            o = 28 * (k // 26) + (k % 26)
            for di in range(3):
                for dj in range(3):
                    p = o + 28 * di + dj
                    if 128 * u <= p < 128 * (u + 1):
                        c1[u, p - 128 * u, m] = conv_w[di, dj]
                    else:
                        c2[u, p - 128 * (u + 1), m] = conv_w[di, dj]

    wp = np.zeros((128, 60), np.float32)
    for u in range(NU):
        hi = min(128 * (u + 1), 676)
        wp[0 : hi - 128 * u, 10 * u : 10 * (u + 1)] = W[128 * u : hi]

    import ml_dtypes

    cpk = np.zeros((128, 1596), np.float32)
    cpk[:, 0:768] = c1.transpose(1, 0, 2).reshape(128, 768)
    cpk[:, 768:1536] = c2.transpose(1, 0, 2).reshape(128, 768)
    cpk[:, 1536:1596] = wp
    return cpk.astype(ml_dtypes.bfloat16), b.reshape(10, 1).copy()


def _pack_x(x):
    """[32768, 784] fp32 -> per-core [1024, 3584] bf16, chunk-major,
    partition-contiguous (row 128n+p, col 512t+c <- chunk n, pixel
    128t+p, sample c; pixels 784..895 zero)."""
    import ml_dtypes

    xb = np.ascontiguousarray(x.astype(ml_dtypes.bfloat16))
    shards = []
    for c in range(N_CORES):
        shard = xb[c * B_CORE : (c + 1) * B_CORE]  # [4096, 784]
        xp = shard.reshape(N_CHUNKS, CHUNK, 784).transpose(0, 2, 1)  # [8,784,512]
        pad = np.zeros((N_CHUNKS, 896, CHUNK), ml_dtypes.bfloat16)
        pad[:, :784] = xp
        # [8, 7, 128, 512] -> [8, 128, 7, 512] -> [1024, 3584]
        xhc = np.ascontiguousarray(
            pad.reshape(N_CHUNKS, NT, 128, CHUNK).transpose(0, 2, 1, 3)
        ).reshape(N_CHUNKS * 128, NT * CHUNK)
        shards.append(xhc)
    return shards


def _run(inputs, trace=False):
    x = np.asarray(inputs["x"], np.float32)
    conv_w = inputs["conv_w"]
    W = inputs["W"]
    b = inputs["b"]

    if "nc" not in _NC_CACHE:
        _NC_CACHE["nc"] = _build_nc()
    nc = _NC_CACHE["nc"]

    cpk, bias = _build_consts(conv_w, W, b)
    shards = _pack_x(x)

    in_maps = [
        {"xh": shards[c], "cpk": cpk, "bias_in": bias} for c in range(N_CORES)
    ]

    res = run_bass_kernel_spmd(
        nc, in_maps, core_ids=list(range(N_CORES)), trace=trace
    )
    out = np.concatenate(
        [np.asarray(res.results[c]["out_t"]).T for c in range(N_CORES)], axis=0
    )
    return out, res


def kernel(**inputs) -> np.ndarray:
    return _run(inputs, trace=False)[0]
